# revision 4
# baseline (speedup 1.0000x reference)
"""Single-head attention on 8 TRN2 NeuronCores, data-parallel over batch.

Per core (one batch element b):
  x_b [2048, 768] f32 -> Q = x Wq, K = x Wk, V = x Wv (head 64)
  scores^T[k, q] = (K^T slice).T @ Q^T / 8 ; E = exp(scores) (no max-sub:
  |scores| <~ 2.5 so exp is safe); out = (E^T' PV with ones row) -> normalize.

Layout strategy (everything contracts over the partition dim):
  - x: SWDGE cast-DMA f32->bf16 into SBUF natural [seq, emb], then X-bar
    DMA-transpose into x^T [emb, seq] (bf16). No TensorE transposes and no
    psum->SBUF copies for x^T.
  - Q^T/K^T computed with duplicated weights [Wq|Wq] so both partition
    halves hold the same 64 rows -> 2x row-tiled score matmuls (K=64
    contraction in row groups 0-1 / 2-3, alternating by k-tile parity).
  - exp on ScalarE in [128, 1024] batches (2 psum banks) to amortize the
    ~352-cycle per-instruction overhead; 1/sqrt(64) folded into the
    activation's free scale.
  - PV uses lhsT = V' = [V, ones] (M=65): psum row 64 accumulates the
    softmax denominator for free.
  - U^T [65, q] tiles are PE-transposed back to natural [q, 65]; col 64's
    reciprocal normalizes via tensor_scalar_mul, then DMA out.
"""

import numpy as np

import concourse.bass as bass
import concourse.tile as tile
from concourse import bacc, mybir
from concourse.bass_utils import run_bass_kernel_spmd
from concourse.masks import make_identity

B, S, D, H = 8, 2048, 768, 64
P = 128
NT = S // P  # 16 seq tiles
NCH = D // P  # 6 emb chunks
QC = 512  # q-chunk width (one psum bank of f32)
NQ = S // QC  # 4 q chunks
N_CORES = 8
F32 = mybir.dt.float32
BF16 = mybir.dt.bfloat16
EXP = mybir.ActivationFunctionType.Exp
SCALE = float(1.0 / np.sqrt(H))


def build_kernel():
    nc = bacc.Bacc("TRN2", num_devices=N_CORES)
    x_ext = nc.declare_dram_parameter("x", [S, D], F32, isOutput=False)
    wk_ext = nc.declare_dram_parameter("Wk", [D, H], F32, isOutput=False)
    wq_ext = nc.declare_dram_parameter("Wq", [D, H], F32, isOutput=False)
    wv_ext = nc.declare_dram_parameter("Wv", [D, H], F32, isOutput=False)
    out_ext = nc.declare_dram_parameter("out", [S, H], F32, isOutput=True)

    with tile.TileContext(nc) as tc:
        _body(nc, tc, x_ext, wq_ext, wk_ext, wv_ext, out_ext)
    nc.compile()
    return nc


def _body(nc, tc, x_ext, wq_ext, wk_ext, wv_ext, out_ext):
    with (
        tc.tile_pool(name="singles", bufs=1) as singles,
        tc.tile_pool(name="xn", bufs=3) as xn_pool,
        tc.tile_pool(name="et", bufs=3) as et_pool,
        tc.tile_pool(name="fin", bufs=4) as fin_pool,
    ):
        ident = singles.tile([P, P], F32)
        make_identity(nc, ident)

        # ---- weights: DMA f32, cast to bf16, duplicate Q/K across halves
        wq_st = singles.tile([P, NCH, H], F32, tag="wst_q")
        wk_st = singles.tile([P, NCH, H], F32, tag="wst_k")
        wv_st = singles.tile([P, NCH, H], F32, tag="wst_v")
        for c in range(NCH):
            nc.sync.dma_start(out=wq_st[:, c, :], in_=wq_ext[c * P:(c + 1) * P, :])
            nc.sync.dma_start(out=wk_st[:, c, :], in_=wk_ext[c * P:(c + 1) * P, :])
            nc.sync.dma_start(out=wv_st[:, c, :], in_=wv_ext[c * P:(c + 1) * P, :])
        wq2 = singles.tile([P, NCH, 2 * H], BF16, tag="wq2")
        wk2 = singles.tile([P, NCH, 2 * H], BF16, tag="wk2")
        wv_sb = singles.tile([P, NCH, H], BF16, tag="wv_sb")
        nc.vector.tensor_copy(wq2[:, :, 0:H], wq_st)
        nc.vector.tensor_copy(wq2[:, :, H:2 * H], wq_st)
        nc.vector.tensor_copy(wk2[:, :, 0:H], wk_st)
        nc.vector.tensor_copy(wk2[:, :, H:2 * H], wk_st)
        nc.vector.tensor_copy(wv_sb, wv_st)

        xt_sb = singles.tile([P, NCH, S], BF16, tag="xt_sb")  # x^T
        qt2 = singles.tile([P, S], BF16, tag="qt2")  # Q^T in both halves
        kt2 = singles.tile([P, S], BF16, tag="kt2")  # K^T in both halves
        vp = singles.tile([P, NT, H + 1], BF16, tag="vp")  # V' = [V, 1]
        nc.vector.memset(vp[:, :, H:H + 1], 1.0)

        # ---- phase 2: cast-DMA in, DMA-transpose, projections per strip
        with (
            tc.tile_pool(name="ps_p", bufs=2, space="PSUM") as psum_p,
            tc.tile_pool(name="ps_v", bufs=2, space="PSUM") as psum_v,
        ):
            for sc in range(NQ):
                for t in range(4):
                    st = sc * 4 + t
                    xn_t = xn_pool.tile([P, D], BF16, name="xn_t")
                    nc.gpsimd.dma_start(
                        out=xn_t, in_=x_ext[st * P:(st + 1) * P, :])
                    nc.sync.dma_start(
                        out=xt_sb[:, :, st * P:(st + 1) * P], in_=xn_t,
                        transpose=True)
                sl = slice(sc * QC, (sc + 1) * QC)
                psqk = psum_p.tile([P, 2, QC], F32, tag="pp")
                for c in range(NCH):
                    nc.tensor.matmul(psqk[:, 0, :], wq2[:, c, :],
                                     xt_sb[:, c, sl],
                                     start=(c == 0), stop=(c == NCH - 1))
                for c in range(NCH):
                    nc.tensor.matmul(psqk[:, 1, :], wk2[:, c, :],
                                     xt_sb[:, c, sl],
                                     start=(c == 0), stop=(c == NCH - 1))
                nc.vector.tensor_copy(qt2[:, sl], psqk[:, 0, :])
                nc.vector.tensor_copy(kt2[:, sl], psqk[:, 1, :])
                psv = psum_v.tile([P, 4, H], F32, tag="pv")
                for t in range(4):
                    st = sc * 4 + t
                    for c in range(NCH):
                        nc.tensor.matmul(
                            psv[:, t, :], xt_sb[:, c, st * P:(st + 1) * P],
                            wv_sb[:, c, :],
                            start=(c == 0), stop=(c == NCH - 1))
                nc.vector.tensor_copy(vp[:, sc * 4:(sc + 1) * 4, 0:H], psv)

        # ---- phase 3: scores^T -> exp -> PV accumulate
        with (
            tc.tile_pool(name="ps_s", bufs=2, space="PSUM") as psum_s,
            tc.tile_pool(name="ps_u", bufs=4, space="PSUM") as psum_u_pool,
        ):
            psum_u = [psum_u_pool.tile([H + 1, QC], F32, tag="pu",
                                       name=f"psum_u{qc}")
                      for qc in range(NQ)]
            for kt in range(NT):
                lo = (kt % 2) * H  # alternate row groups for concurrency
                ksl = slice(kt * P, (kt + 1) * P)
                et = et_pool.tile([P, S], BF16, name="et")
                for qh in range(2):
                    ps = psum_s.tile([P, 2, QC], F32, tag="ss", name="ps")
                    for j in range(2):
                        qc = qh * 2 + j
                        nc.tensor.matmul(
                            ps[:, j, :], kt2[lo:lo + H, ksl],
                            qt2[lo:lo + H, qc * QC:(qc + 1) * QC],
                            start=True, stop=True)
                    nc.scalar.activation(
                        et[:, qh * 2 * QC:(qh + 1) * 2 * QC].rearrange(
                            "p (a b) -> p a b", b=QC),
                        ps, EXP, scale=SCALE)
                for qc in range(NQ):
                    nc.tensor.matmul(
                        psum_u[qc], vp[:, kt, :],
                        et[:, qc * QC:(qc + 1) * QC],
                        start=(kt == 0), stop=(kt == NT - 1))

            # ---- phase 4: transpose U^T back, normalize, DMA out
            for qc in range(NQ):
                ut = fin_pool.tile([H + 1, QC], F32, tag="ut", name="ut")
                nc.scalar.copy(out=ut, in_=psum_u[qc])
                for t in range(4):
                    qt = qc * 4 + t
                    pso = psum_s.tile([P, H + 1], F32, tag="ss", name="pso")
                    nc.tensor.transpose(
                        pso, ut[:, t * P:(t + 1) * P], ident[:H + 1, :H + 1])
                    rcp = fin_pool.tile([P, 1], F32, tag="rcp", name="rcp")
                    nc.vector.reciprocal(rcp, pso[:, H:H + 1])
                    ot = fin_pool.tile([P, H], F32, tag="ot", name="ot")
                    nc.vector.tensor_scalar_mul(ot, pso[:, 0:H], rcp)
                    nc.sync.dma_start(
                        out=out_ext[qt * P:(qt + 1) * P, :], in_=ot)


_cached_nc = None


def kernel(**inputs):
    global _cached_nc
    x = np.ascontiguousarray(inputs["x"], dtype=np.float32)
    wk = np.ascontiguousarray(inputs["Wk"], dtype=np.float32)
    wq = np.ascontiguousarray(inputs["Wq"], dtype=np.float32)
    wv = np.ascontiguousarray(inputs["Wv"], dtype=np.float32)
    assert x.shape == (B, S, D)

    if _cached_nc is None:
        _cached_nc = build_kernel()
    nc = _cached_nc

    in_maps = [{"x": x[b], "Wk": wk, "Wq": wq, "Wv": wv} for b in range(B)]
    res = run_bass_kernel_spmd(nc, in_maps, core_ids=list(range(N_CORES)))
    return np.stack([res.results[i]["out"] for i in range(N_CORES)], axis=0)


# revision 7
# speedup vs baseline: 1.2004x; 1.2004x over previous
"""Single-head attention on 8 TRN2 NeuronCores, data-parallel over batch.

Per core (one batch element b):
  x_b [2048, 768] f32 -> Q = x Wq, K = x Wk, V = x Wv (head 64)
  scores^T[k, q] = (K^T slice).T @ Q^T / 8 ; E = exp(scores) (no max-sub:
  |scores| <~ 2.5 so exp is safe); out = (E^T' PV with ones row) -> normalize.

Layout strategy (everything contracts over the partition dim):
  - x: SWDGE cast-DMA f32->bf16 into SBUF natural [seq, emb], then X-bar
    DMA-transpose into x^T [emb, seq] (bf16). No TensorE transposes and no
    psum->SBUF copies for x^T.
  - Q^T/K^T computed with duplicated weights [Wq|Wq] so both partition
    halves hold the same 64 rows -> 2x row-tiled score matmuls (K=64
    contraction in row groups 0-1 / 2-3, alternating by k-tile parity).
  - exp on ScalarE in [128, 1024] batches (2 psum banks) to amortize the
    ~352-cycle per-instruction overhead; 1/sqrt(64) folded into the
    activation's free scale.
  - PV uses lhsT = V' = [V, ones] (M=65): psum row 64 accumulates the
    softmax denominator for free.
  - U^T [65, q] tiles are PE-transposed back to natural [q, 65]; col 64's
    reciprocal normalizes via tensor_scalar_mul, then DMA out.
"""

import numpy as np

import concourse.bass as bass
import concourse.tile as tile
from concourse import bacc, mybir
from concourse.bass_utils import run_bass_kernel_spmd
from concourse.masks import make_identity

B, S, D, H = 8, 2048, 768, 64
P = 128
NT = S // P  # 16 seq tiles
NCH = D // P  # 6 emb chunks
QC = 512  # q-chunk width (one psum bank of f32)
NQ = S // QC  # 4 q chunks
N_CORES = 8
F32 = mybir.dt.float32
BF16 = mybir.dt.bfloat16
EXP = mybir.ActivationFunctionType.Exp
SCALE = float(1.0 / np.sqrt(H))


def build_kernel():
    nc = bacc.Bacc("TRN2", num_devices=N_CORES)
    x_ext = nc.declare_dram_parameter("x", [S, D], F32, isOutput=False)
    wk_ext = nc.declare_dram_parameter("Wk", [D, H], F32, isOutput=False)
    wq_ext = nc.declare_dram_parameter("Wq", [D, H], F32, isOutput=False)
    wv_ext = nc.declare_dram_parameter("Wv", [D, H], F32, isOutput=False)
    out_ext = nc.declare_dram_parameter("out", [S, H], F32, isOutput=True)

    with tile.TileContext(nc) as tc:
        _body(nc, tc, x_ext, wq_ext, wk_ext, wv_ext, out_ext)
    nc.compile()
    return nc


def _body(nc, tc, x_ext, wq_ext, wk_ext, wv_ext, out_ext):
    with (
        tc.tile_pool(name="singles", bufs=1) as singles,
        tc.tile_pool(name="xn", bufs=3) as xn_pool,
        tc.tile_pool(name="et", bufs=3) as et_pool,
        tc.tile_pool(name="fin", bufs=4) as fin_pool,
    ):
        ident = singles.tile([P, P], F32)
        make_identity(nc, ident)

        # ---- weights: DMA f32, cast to bf16, duplicate Q/K across halves
        wq_st = singles.tile([P, NCH, H], F32, tag="wst_q")
        wk_st = singles.tile([P, NCH, H], F32, tag="wst_k")
        wv_st = singles.tile([P, NCH, H], F32, tag="wst_v")
        for c in range(NCH):
            nc.sync.dma_start(out=wq_st[:, c, :], in_=wq_ext[c * P:(c + 1) * P, :])
            nc.sync.dma_start(out=wk_st[:, c, :], in_=wk_ext[c * P:(c + 1) * P, :])
            nc.sync.dma_start(out=wv_st[:, c, :], in_=wv_ext[c * P:(c + 1) * P, :])
        wq2 = singles.tile([P, NCH, 2 * H], BF16, tag="wq2")
        wk2 = singles.tile([P, NCH, 2 * H], BF16, tag="wk2")
        wv_sb = singles.tile([P, NCH, H], BF16, tag="wv_sb")
        nc.vector.tensor_copy(wq2[:, :, 0:H], wq_st)
        nc.vector.tensor_copy(wq2[:, :, H:2 * H], wq_st)
        nc.vector.tensor_copy(wk2[:, :, 0:H], wk_st)
        nc.vector.tensor_copy(wk2[:, :, H:2 * H], wk_st)
        nc.vector.tensor_copy(wv_sb, wv_st)

        xt_sb = singles.tile([P, NCH, S], BF16, tag="xt_sb")  # x^T
        qt2 = singles.tile([P, S], BF16, tag="qt2")  # Q^T in both halves
        kt2 = singles.tile([P, S], BF16, tag="kt2")  # K^T in both halves
        vp = singles.tile([P, NT, H + 1], BF16, tag="vp")  # V' = [V, 1]
        nc.vector.memset(vp[:, :, H:H + 1], 1.0)

        # ---- phase 2: cast-DMA in, DMA-transpose, Q/K projections per strip
        # (V projections are deferred into phase 3 as PE gap-filler.)
        with (
            tc.tile_pool(name="ps_s", bufs=2, space="PSUM") as psum_s,
            tc.tile_pool(name="ps_u", bufs=3, space="PSUM") as psum_u_pool,
            tc.tile_pool(name="ps_v", bufs=1, space="PSUM") as psum_v,
        ):
            psum_p = psum_s  # QK-proj tiles share the score pool slots
            for sc in range(NQ):
                for t in range(4):
                    st = sc * 4 + t
                    xn_t = xn_pool.tile([P, D], BF16, name="xn_t")
                    nc.gpsimd.dma_start(
                        out=xn_t, in_=x_ext[st * P:(st + 1) * P, :])
                    nc.sync.dma_start(
                        out=xt_sb[:, :, st * P:(st + 1) * P], in_=xn_t,
                        transpose=True)
                sl = slice(sc * QC, (sc + 1) * QC)
                psqk = psum_p.tile([P, 2, QC], F32, tag="ss", name="psqk")
                for c in range(NCH):
                    nc.tensor.matmul(psqk[:, 0, :], wq2[:, c, :],
                                     xt_sb[:, c, sl],
                                     start=(c == 0), stop=(c == NCH - 1))
                for c in range(NCH):
                    nc.tensor.matmul(psqk[:, 1, :], wk2[:, c, :],
                                     xt_sb[:, c, sl],
                                     start=(c == 0), stop=(c == NCH - 1))
                nc.vector.tensor_copy(qt2[:, sl], psqk[:, 0, :])
                nc.vector.tensor_copy(kt2[:, sl], psqk[:, 1, :])

            # ---- phase 3: per q-half: scores^T -> exp -> PV accumulate
            for qh in range(2):
                psum_u = [psum_u_pool.tile([H + 1, QC], F32, tag="pu",
                                           name=f"psum_u{qh}_{j}")
                          for j in range(2)]
                for kt in range(NT):
                    ksl = slice(kt * P, (kt + 1) * P)
                    if qh == 0:
                        # just-in-time V projection for this k-tile
                        psv = psum_v.tile([P, H], F32, tag="pv", name="psv")
                        for c in range(NCH):
                            nc.tensor.matmul(
                                psv, xt_sb[:, c, ksl], wv_sb[:, c, :],
                                start=(c == 0), stop=(c == NCH - 1))
                        nc.vector.tensor_copy(vp[:, kt, 0:H], psv)
                    et = et_pool.tile([P, 2 * QC], BF16, name="et")
                    ps = psum_s.tile([P, 2, QC], F32, tag="ss", name="ps")
                    for j in range(2):
                        lo = j * H  # row groups 0-1 / 2-3 run concurrently
                        qc = qh * 2 + j
                        nc.tensor.matmul(
                            ps[:, j, :], kt2[lo:lo + H, ksl],
                            qt2[lo:lo + H, qc * QC:(qc + 1) * QC],
                            start=True, stop=True)
                    nc.scalar.activation(
                        et.rearrange("p (a b) -> p a b", b=QC),
                        ps, EXP, scale=SCALE)
                    for j in range(2):
                        nc.tensor.matmul(
                            psum_u[j], vp[:, kt, :],
                            et[:, j * QC:(j + 1) * QC],
                            start=(kt == 0), stop=(kt == NT - 1))

                # ---- phase 4 (per half): transpose U^T, normalize, DMA out
                for j in range(2):
                    qc = qh * 2 + j
                    ut = fin_pool.tile([H + 1, QC], F32, tag="ut", name="ut")
                    nc.scalar.copy(out=ut, in_=psum_u[j])
                    for t in range(4):
                        qt = qc * 4 + t
                        pso = psum_v.tile([P, H + 1], F32, tag="pv",
                                          name="pso")
                        nc.tensor.transpose(
                            pso, ut[:, t * P:(t + 1) * P],
                            ident[:H + 1, :H + 1])
                        rcp = fin_pool.tile([P, 1], F32, tag="rcp",
                                            name="rcp")
                        nc.vector.reciprocal(rcp, pso[:, H:H + 1])
                        ot = fin_pool.tile([P, H], F32, tag="ot", name="ot")
                        nc.vector.tensor_scalar_mul(ot, pso[:, 0:H], rcp)
                        nc.sync.dma_start(
                            out=out_ext[qt * P:(qt + 1) * P, :], in_=ot)


_cached_nc = None


def kernel(**inputs):
    global _cached_nc
    x = np.ascontiguousarray(inputs["x"], dtype=np.float32)
    wk = np.ascontiguousarray(inputs["Wk"], dtype=np.float32)
    wq = np.ascontiguousarray(inputs["Wq"], dtype=np.float32)
    wv = np.ascontiguousarray(inputs["Wv"], dtype=np.float32)
    assert x.shape == (B, S, D)

    if _cached_nc is None:
        _cached_nc = build_kernel()
    nc = _cached_nc

    in_maps = [{"x": x[b], "Wk": wk, "Wq": wq, "Wv": wv} for b in range(B)]
    res = run_bass_kernel_spmd(nc, in_maps, core_ids=list(range(N_CORES)))
    return np.stack([res.results[i]["out"] for i in range(N_CORES)], axis=0)


# revision 8
# speedup vs baseline: 1.6314x; 1.3591x over previous
"""Single-head attention on 8 TRN2 NeuronCores, data-parallel over batch.

Per core (one batch element b):
  x_b [2048, 768] f32 -> Q = x Wq, K = x Wk, V = x Wv (head 64)
  scores^T[k, q] = (K^T slice).T @ Q^T / 8 ; E = exp(scores) (no max-sub:
  |scores| <~ 2.5 so exp is safe); out = (E^T' PV with ones row) -> normalize.

Layout strategy (everything contracts over the partition dim):
  - x: SWDGE cast-DMA f32->bf16 into SBUF natural [seq, emb], then X-bar
    DMA-transpose into x^T [emb, seq] (bf16). No TensorE transposes and no
    psum->SBUF copies for x^T.
  - Q^T/K^T computed with duplicated weights [Wq|Wq] so both partition
    halves hold the same 64 rows -> 2x row-tiled score matmuls (K=64
    contraction in row groups 0-1 / 2-3, alternating by k-tile parity).
  - exp on ScalarE in [128, 1024] batches (2 psum banks) to amortize the
    ~352-cycle per-instruction overhead; 1/sqrt(64) folded into the
    activation's free scale.
  - PV uses lhsT = V' = [V, ones] (M=65): psum row 64 accumulates the
    softmax denominator for free.
  - U^T [65, q] tiles are PE-transposed back to natural [q, 65]; col 64's
    reciprocal normalizes via tensor_scalar_mul, then DMA out.
"""

import numpy as np

import concourse.bass as bass
import concourse.tile as tile
from concourse import bacc, mybir
from concourse.bass_utils import run_bass_kernel_spmd
from concourse.masks import make_identity

B, S, D, H = 8, 2048, 768, 64
P = 128
NT = S // P  # 16 seq tiles
NCH = D // P  # 6 emb chunks
QC = 512  # q-chunk width (one psum bank of f32)
NQ = S // QC  # 4 q chunks
N_CORES = 8
F32 = mybir.dt.float32
BF16 = mybir.dt.bfloat16
EXP = mybir.ActivationFunctionType.Exp
SCALE = float(1.0 / np.sqrt(H))


def build_kernel():
    nc = bacc.Bacc("TRN2", num_devices=N_CORES)
    x_ext = nc.declare_dram_parameter("x", [S, D], F32, isOutput=False)
    wk_ext = nc.declare_dram_parameter("Wk", [D, H], F32, isOutput=False)
    wq_ext = nc.declare_dram_parameter("Wq", [D, H], F32, isOutput=False)
    wv_ext = nc.declare_dram_parameter("Wv", [D, H], F32, isOutput=False)
    out_ext = nc.declare_dram_parameter("out", [S, H], F32, isOutput=True)

    with tile.TileContext(nc) as tc:
        _body(nc, tc, x_ext, wq_ext, wk_ext, wv_ext, out_ext)
    nc.compile()
    return nc


def _body(nc, tc, x_ext, wq_ext, wk_ext, wv_ext, out_ext):
    with (
        tc.tile_pool(name="singles", bufs=1) as singles,
        tc.tile_pool(name="xn", bufs=3) as xn_pool,
        tc.tile_pool(name="et", bufs=3) as et_pool,
        tc.tile_pool(name="fin", bufs=4) as fin_pool,
    ):
        ident = singles.tile([P, P], F32)
        make_identity(nc, ident)

        # ---- weights: DMA f32, cast to bf16, duplicate Q/K across halves
        wq_st = singles.tile([P, NCH, H], F32, tag="wst_q")
        wk_st = singles.tile([P, NCH, H], F32, tag="wst_k")
        wv_st = singles.tile([P, NCH, H], F32, tag="wst_v")
        for c in range(NCH):
            nc.sync.dma_start(out=wq_st[:, c, :], in_=wq_ext[c * P:(c + 1) * P, :])
            nc.sync.dma_start(out=wk_st[:, c, :], in_=wk_ext[c * P:(c + 1) * P, :])
            nc.sync.dma_start(out=wv_st[:, c, :], in_=wv_ext[c * P:(c + 1) * P, :])
        wq2 = singles.tile([P, NCH, 2 * H], BF16, tag="wq2")
        wk2 = singles.tile([P, NCH, 2 * H], BF16, tag="wk2")
        wv_sb = singles.tile([P, NCH, H], BF16, tag="wv_sb")
        nc.vector.tensor_copy(wq2[:, :, 0:H], wq_st)
        nc.vector.tensor_copy(wq2[:, :, H:2 * H], wq_st)
        nc.vector.tensor_copy(wk2[:, :, 0:H], wk_st)
        nc.vector.tensor_copy(wk2[:, :, H:2 * H], wk_st)
        nc.vector.tensor_copy(wv_sb, wv_st)

        xt_sb = singles.tile([P, NCH, S], BF16, tag="xt_sb")  # x^T
        qt2 = singles.tile([P, S], BF16, tag="qt2")  # Q^T in both halves
        kt2 = singles.tile([P, S], BF16, tag="kt2")  # K^T in both halves
        vp = singles.tile([P, NT, H + 1], BF16, tag="vp")  # V' = [V, 1]
        nc.vector.memset(vp[:, :, H:H + 1], 1.0)

        # ---- phase 2: cast-DMA in, DMA-transpose, Q/K projections per strip
        # (V projections are deferred into phase 3 as PE gap-filler.)
        with (
            tc.tile_pool(name="ps_s", bufs=2, space="PSUM") as psum_s,
            tc.tile_pool(name="ps_u", bufs=3, space="PSUM") as psum_u_pool,
            tc.tile_pool(name="ps_v", bufs=1, space="PSUM") as psum_v,
        ):
            psum_p = psum_s  # QK-proj tiles share the score pool slots
            # all cast-DMAs first (SWDGE latency ~12us amortizes across 16),
            # then all DMA-transposes, then projections per strip
            xn_tiles = [xn_pool.tile([P, D], BF16, name=f"xn_{st}",
                                     tag=f"xn_{st}")
                        for st in range(NT)]
            for st in range(NT):
                nc.gpsimd.dma_start(
                    out=xn_tiles[st], in_=x_ext[st * P:(st + 1) * P, :])
            for st in range(NT):
                nc.sync.dma_start(
                    out=xt_sb[:, :, st * P:(st + 1) * P], in_=xn_tiles[st],
                    transpose=True)
            for sc in range(NQ):
                sl = slice(sc * QC, (sc + 1) * QC)
                psqk = psum_p.tile([P, 2, QC], F32, tag="ss", name="psqk")
                for c in range(NCH):
                    nc.tensor.matmul(psqk[:, 0, :], wq2[:, c, :],
                                     xt_sb[:, c, sl],
                                     start=(c == 0), stop=(c == NCH - 1))
                for c in range(NCH):
                    nc.tensor.matmul(psqk[:, 1, :], wk2[:, c, :],
                                     xt_sb[:, c, sl],
                                     start=(c == 0), stop=(c == NCH - 1))
                nc.vector.tensor_copy(qt2[:, sl], psqk[:, 0, :])
                nc.vector.tensor_copy(kt2[:, sl], psqk[:, 1, :])

            # ---- phase 3: per q-half: scores^T -> exp -> PV accumulate
            for qh in range(2):
                psum_u = [psum_u_pool.tile([H + 1, QC], F32, tag="pu",
                                           name=f"psum_u{qh}_{j}")
                          for j in range(2)]
                for kt in range(NT):
                    ksl = slice(kt * P, (kt + 1) * P)
                    if qh == 0:
                        # just-in-time V projection for this k-tile
                        psv = psum_v.tile([P, H], F32, tag="pv", name="psv")
                        for c in range(NCH):
                            nc.tensor.matmul(
                                psv, xt_sb[:, c, ksl], wv_sb[:, c, :],
                                start=(c == 0), stop=(c == NCH - 1))
                        nc.vector.tensor_copy(vp[:, kt, 0:H], psv)
                    et = et_pool.tile([P, 2 * QC], BF16, name="et")
                    ps = psum_s.tile([P, 2, QC], F32, tag="ss", name="ps")
                    for j in range(2):
                        lo = j * H  # row groups 0-1 / 2-3 run concurrently
                        qc = qh * 2 + j
                        nc.tensor.matmul(
                            ps[:, j, :], kt2[lo:lo + H, ksl],
                            qt2[lo:lo + H, qc * QC:(qc + 1) * QC],
                            start=True, stop=True)
                    nc.scalar.activation(
                        et.rearrange("p (a b) -> p a b", b=QC),
                        ps, EXP, scale=SCALE)
                    for j in range(2):
                        nc.tensor.matmul(
                            psum_u[j], vp[:, kt, :],
                            et[:, j * QC:(j + 1) * QC],
                            start=(kt == 0), stop=(kt == NT - 1))

                # ---- phase 4 (per half): transpose U^T, normalize, DMA out
                for j in range(2):
                    qc = qh * 2 + j
                    ut = fin_pool.tile([H + 1, QC], F32, tag="ut", name="ut")
                    nc.scalar.copy(out=ut, in_=psum_u[j])
                    for t in range(4):
                        qt = qc * 4 + t
                        pso = psum_v.tile([P, H + 1], F32, tag="pv",
                                          name="pso")
                        nc.tensor.transpose(
                            pso, ut[:, t * P:(t + 1) * P],
                            ident[:H + 1, :H + 1])
                        rcp = fin_pool.tile([P, 1], F32, tag="rcp",
                                            name="rcp")
                        nc.vector.reciprocal(rcp, pso[:, H:H + 1])
                        ot = fin_pool.tile([P, H], F32, tag="ot", name="ot")
                        nc.vector.tensor_scalar_mul(ot, pso[:, 0:H], rcp)
                        nc.sync.dma_start(
                            out=out_ext[qt * P:(qt + 1) * P, :], in_=ot)


_cached_nc = None


def kernel(**inputs):
    global _cached_nc
    x = np.ascontiguousarray(inputs["x"], dtype=np.float32)
    wk = np.ascontiguousarray(inputs["Wk"], dtype=np.float32)
    wq = np.ascontiguousarray(inputs["Wq"], dtype=np.float32)
    wv = np.ascontiguousarray(inputs["Wv"], dtype=np.float32)
    assert x.shape == (B, S, D)

    if _cached_nc is None:
        _cached_nc = build_kernel()
    nc = _cached_nc

    in_maps = [{"x": x[b], "Wk": wk, "Wq": wq, "Wv": wv} for b in range(B)]
    res = run_bass_kernel_spmd(nc, in_maps, core_ids=list(range(N_CORES)))
    return np.stack([res.results[i]["out"] for i in range(N_CORES)], axis=0)
